# revision 1
# baseline (speedup 1.0000x reference)
"""Clustered Linformer Attention — Trainium2 Bass kernel, 8 NeuronCores.

Strategy: data-parallel over batch (2 batches/core, no collectives).
Math restructuring (verified vs reference to ~7e-7 in f32):
  - mask is all-ones => cluster c holds positions [32c, 32c+32); the per-head
    gather+einsum projections become  k_proj = AE[h]^T @ k_h  with a host-built
    sparse table AE[h] in [S, P] (score scale folded in), same for v with AF.
  - the 3-kernel conv fusion over scores collapses to 5 "tap" matrices M_t in
    [P, P] (t in -2..2):  scores_conv[s] = sum_t  (q[s+t] @ (k_proj^T @ M_t)).
    Taps are applied as 5 PSUM-accumulated matmuls with a column-shifted
    (zero-padded) q^T operand.
  - adjacent heads are packed block-diagonally so every matmul contracts over
    the full 128 partitions.
  - softmax has no max-subtraction (|scores| <~ 1.6, exp is safe in f32);
    Z = sum_c exp is computed by an all-ones block-diag matmul that also
    broadcasts Z to all 128 partitions, so normalization is one DVE op.
"""
import sys
import numpy as np
import ml_dtypes

sys.path.insert(0, '/opt/trn_rl_repo')

B, S, D = 16, 2048, 512
H, P, C = 8, 64, 32
DEPTH = D // H           # 64
NCORES = 8
BLOC = B // NCORES       # 2 batches per core
NPAIR = H // 2           # 4 head pairs
SCH = 4                  # s-chunks of 512
SCW = S // SCH           # 512
NJ = S // 128            # 16 s-tiles of 128
NDC = D // 128           # 4 contraction chunks

_CACHE = {}


def _build_nc():
    import concourse.tile as tile
    from concourse import mybir, bacc

    f32 = mybir.dt.float32
    f32r = mybir.dt.float32r
    bf16 = mybir.dt.bfloat16

    nc = bacc.Bacc()
    xT = nc.declare_dram_parameter("xT", [BLOC, D, S], bf16, isOutput=False)
    wq = nc.declare_dram_parameter("wq", [D, D], bf16, isOutput=False)
    wk = nc.declare_dram_parameter("wk", [D, D], bf16, isOutput=False)
    wv = nc.declare_dram_parameter("wv", [D, D], bf16, isOutput=False)
    dw = nc.declare_dram_parameter("dw", [D, D], bf16, isOutput=False)
    dbb = nc.declare_dram_parameter("dbb", [1, D], bf16, isOutput=False)
    ae = nc.declare_dram_parameter("ae", [NPAIR, S, 128], bf16, isOutput=False)
    af = nc.declare_dram_parameter("af", [NPAIR, S, 128], bf16, isOutput=False)
    bdm = nc.declare_dram_parameter("bdm", [5, 128, 128], bf16, isOutput=False)
    onesbd = nc.declare_dram_parameter("onesbd", [128, 128], bf16, isOutput=False)
    out = nc.declare_dram_parameter("out", [BLOC, S, D], f32, isOutput=True)

    import concourse.bass as bass

    with tile.TileContext(nc) as tc:
        with tc.tile_pool(name="const", bufs=1) as cpool, \
             tc.tile_pool(name="big", bufs=1) as bigp, \
             tc.tile_pool(name="ah", bufs=2) as ahp, \
             tc.tile_pool(name="sm", bufs=4) as smp, \
             tc.tile_pool(name="bd", bufs=4) as bdp, \
             tc.tile_pool(name="ob", bufs=3) as obp, \
             tc.tile_pool(name="psB", bufs=6, space="PSUM") as psB, \
             tc.tile_pool(name="psS", bufs=2, space="PSUM") as psS:

            # ---- constants in SBUF ----
            wq_sb = cpool.tile([128, NDC, D], bf16)
            wk_sb = cpool.tile([128, NDC, D], bf16)
            wv_sb = cpool.tile([128, NDC, D], bf16)
            dw_sb = cpool.tile([128, NDC, D], bf16)
            for t_sb, t_dr in ((wk_sb, wk), (wv_sb, wv), (wq_sb, wq), (dw_sb, dw)):
                nc.sync.dma_start(out=t_sb, in_=t_dr[:].rearrange("(o p) m -> p o m", p=128))
            bdm_sb = cpool.tile([128, 5, 128], bf16)
            nc.sync.dma_start(out=bdm_sb, in_=bdm[:].rearrange("t p m -> p t m"))
            ones_sb = cpool.tile([128, 128], bf16)
            nc.sync.dma_start(out=ones_sb, in_=onesbd[:])
            # dense bias: applied as a k=1 accumulating matmul
            # (ones[1,128]^T @ bias_row[1,512]) so no DVE add is needed.
            bias_row = cpool.tile([1, D], bf16)
            nc.sync.dma_start(out=bias_row, in_=dbb[:])
            ones_col = cpool.tile([1, 128], bf16)
            nc.vector.memset(ones_col, 1.0)

            # Per-batch state; stages are emitted as closures so the two
            # batches can be interleaved in PE program order (engines execute
            # in order -- without interleaving, batch 1's QKV sits behind
            # batch 0's softmax gaps instead of filling them).
            st = [dict() for _ in range(BLOC)]

            def emit_x_load(b):
                s = st[b]
                s["xt"] = [bigp.tile([128, S], bf16, tag="xt", bufs=4,
                                     name=f"xt_{b}_{dc}")
                           for dc in range(NDC)]
                for dc in range(NDC):
                    nc.gpsimd.dma_start(
                        out=s["xt"][dc],
                        in_=xT[b, 128 * dc:128 * (dc + 1), :])

            def emit_kv(b, j):
                s = st[b]
                if j == 0:
                    s["knat"] = bigp.tile([128, NJ, D], bf16, tag="knat",
                                          name=f"knat_{b}")
                    s["vnat"] = bigp.tile([128, NJ, D], bf16, tag="vnat",
                                          name=f"vnat_{b}")
                for w_sb, key in ((wk_sb, "knat"), (wv_sb, "vnat")):
                    ps_k = psB.tile([128, D], f32, tag="ps512")
                    for dc in range(NDC):
                        nc.tensor.matmul(
                            ps_k,
                            s["xt"][dc][:, 128 * j:128 * (j + 1)],
                            w_sb[:, dc, :],
                            start=(dc == 0), stop=(dc == NDC - 1))
                    if key == "knat":
                        nc.vector.tensor_copy(out=s[key][:, j, :], in_=ps_k)
                    else:
                        nc.scalar.copy(out=s[key][:, j, :], in_=ps_k)

            def emit_qt(b, pr, n):
                s = st[b]
                if pr == 0 and n == 0:
                    s["qt"] = bigp.tile([128, NPAIR, SCW * SCH + 4], bf16,
                                        tag="qT", bufs=2, name=f"qt_{b}")
                    nc.vector.memset(s["qt"][:, :, 0:2], 0.0)
                    nc.vector.memset(s["qt"][:, :, SCW * SCH + 2:], 0.0)
                ps_q = psB.tile([128, SCW], f32, tag="ps512")
                for dc in range(NDC):
                    nc.tensor.matmul(
                        ps_q,
                        wq_sb[:, dc, 128 * pr:128 * (pr + 1)],
                        s["xt"][dc][:, SCW * n:SCW * (n + 1)],
                        start=(dc == 0), stop=(dc == NDC - 1))
                nc.scalar.copy(
                    out=s["qt"][:, pr, 2 + SCW * n:2 + SCW * (n + 1)],
                    in_=ps_q)

            def emit_proj(b, pr):
                s = st[b]
                if pr == 0:
                    s["kp"] = bigp.tile([128, NPAIR, 128], bf16, tag="kpbd",
                                        bufs=2, name=f"kp_{b}")
                    s["vp"] = bigp.tile([128, NPAIR, 128], bf16, tag="vpbd",
                                        bufs=2, name=f"vp_{b}")
                    nc.vector.memset(s["kp"], 0.0)
                    nc.vector.memset(s["vp"], 0.0)
                ae_sb = ahp.tile([128, NJ, 128], bf16, tag="ae")
                af_sb = ahp.tile([128, NJ, 128], bf16, tag="af")
                nc.sync.dma_start(out=ae_sb, in_=ae[pr].rearrange("(j p) c -> p j c", p=128))
                nc.sync.dma_start(out=af_sb, in_=af[pr].rearrange("(j p) c -> p j c", p=128))
                for a_sb, key, dstk in ((ae_sb, "knat", "kp"), (af_sb, "vnat", "vp")):
                    # lhsT = [A_h0 | A_h1] columns, rhs = both heads' k/v
                    # columns; out diag blocks = the two k_proj's, off-diag
                    # blocks are cross-head garbage and are not copied.
                    ps_p = psS.tile([128, 128], f32, tag="pssmall")
                    for j in range(NJ):
                        nc.tensor.matmul(
                            ps_p,
                            a_sb[:, j, :],
                            st[b][key][:, j, 128 * pr:128 * (pr + 1)],
                            start=(j == 0), stop=(j == NJ - 1))
                    dst = st[b][dstk]
                    nc.vector.tensor_copy(
                        out=dst[0:64, pr, 0:64], in_=ps_p[0:64, 0:64])
                    nc.vector.tensor_copy(
                        out=dst[64:128, pr, 64:128], in_=ps_p[64:128, 64:128])

            def emit_kt(b, pr):
                s = st[b]
                if pr == 0:
                    s["concat"] = bigp.tile([128, NPAIR, S], bf16,
                                            tag="concatT", bufs=2,
                                            name=f"concat_{b}")
                    s["bdt"] = {}
                bdt = bdp.tile([128, 5, 128], bf16, tag="bdt",
                               name=f"bdt_{b}_{pr}")
                s["bdt"][pr] = bdt
                for t in range(5):
                    ps_b = psS.tile([128, 128], f32, tag="pssmall")
                    nc.tensor.matmul(ps_b, s["kp"][:, pr, :], bdm_sb[:, t, :],
                                     start=True, stop=True)
                    nc.vector.tensor_copy(out=bdt[:, t, :], in_=ps_b)

            def emit_att(b, pr, n):
                s = st[b]
                bdt = s["bdt"][pr]
                ps_sc = psB.tile([128, SCW], f32, tag="ps512")
                for ti in range(5):  # t = ti - 2
                    nc.tensor.matmul(
                        ps_sc,
                        bdt[:, ti, :],
                        s["qt"][:, pr, SCW * n + ti:SCW * n + ti + SCW],
                        start=(ti == 0), stop=(ti == 4))
                expt = smp.tile([128, SCW], bf16, tag="expt")
                nc.scalar.activation(
                    out=expt, in_=ps_sc,
                    func=mybir.ActivationFunctionType.Exp)
                ps_z = psB.tile([128, SCW], f32, tag="ps512")
                nc.tensor.matmul(ps_z, ones_sb, expt, start=True, stop=True)
                ps_at = psB.tile([128, SCW], f32, tag="ps512")
                nc.tensor.matmul(ps_at, vp_bd_of(s)[:, pr, :], expt,
                                 start=True, stop=True)
                # 1/Z: approx reciprocal (~18 bits, single DVE op). Exact
                # reciprocal is ~3.3us/tile; ACT ln/exp thrashes the table.
                rzb = smp.tile([128, SCW], f32, tag="rzb")
                nc.vector.reciprocal_approx_fast(out=rzb, in_=ps_z)
                nc.vector.tensor_mul(
                    out=s["concat"][:, pr, SCW * n:SCW * (n + 1)],
                    in0=ps_at, in1=rzb)

            def vp_bd_of(s):
                return s["vp"]

            def emit_dense(b, j):
                s = st[b]
                ps_d = psB.tile([128, D], f32, tag="ps512")
                for dc in range(NDC):
                    nc.tensor.matmul(
                        ps_d,
                        s["concat"][:, dc, 128 * j:128 * (j + 1)],
                        dw_sb[:, dc, :],
                        start=(dc == 0), stop=False)
                nc.tensor.matmul(ps_d, ones_col, bias_row,
                                 start=False, stop=True)
                obuf = obp.tile([128, D], f32, tag="obuf")
                nc.scalar.copy(out=obuf, in_=ps_d)
                nc.sync.dma_start(out=out[b, 128 * j:128 * (j + 1), :],
                                  in_=obuf)

            # ---- emission schedule ----
            emit_x_load(0)
            for j in range(NJ):
                emit_kv(0, j)
            for pr in range(NPAIR):
                for n in range(SCH):
                    emit_qt(0, pr, n)
            for pr in range(NPAIR):
                emit_proj(0, pr)
            emit_x_load(1)
            # batch-0 attention interleaved with batch-1 kv+qT
            fillers = [(emit_kv, (1, j)) for j in range(NJ)] + \
                      [(emit_qt, (1, pr, n)) for pr in range(NPAIR)
                       for n in range(SCH)]
            fi = 0
            for pr in range(NPAIR):
                emit_kt(0, pr)
                for n in range(SCH):
                    emit_att(0, pr, n)
                    for _ in range(2):
                        if fi < len(fillers):
                            f, a = fillers[fi]; f(*a); fi += 1
            while fi < len(fillers):
                f, a = fillers[fi]; f(*a); fi += 1
            for pr in range(NPAIR):
                emit_proj(1, pr)
            # batch-1 attention (n-outer) interleaved with batch-0 dense;
            # once chunk n is done for all pairs, batch-1 dense joins too.
            for pr in range(NPAIR):
                emit_kt(1, pr)
            dj0 = 0
            for n in range(SCH):
                for pr in range(NPAIR):
                    emit_att(1, pr, n)
                    if dj0 < NJ:
                        emit_dense(0, dj0); dj0 += 1
                if n > 0:
                    for j in range(SCH * (n - 1), SCH * n):
                        emit_dense(1, j)
            while dj0 < NJ:
                emit_dense(0, dj0); dj0 += 1
            for j in range(SCH * (SCH - 1), NJ):
                emit_dense(1, j)
            for j in range(0, SCH * (SCH - 1)):
                pass  # emitted above


    nc.finalize()
    return nc


def _prep_inputs(x, mask, wq, wk, wv, EW, FW, conv_w1, conv_w3, conv_w5, conv_b,
                 dense_w, dense_b, cluster_table):
    """Host-side restructuring -> per-core input maps."""
    bf = ml_dtypes.bfloat16
    x = np.ascontiguousarray(np.asarray(x, np.float32))
    mask = np.asarray(mask)
    counts = np.clip(mask.astype(np.int64).sum(1), 1, S)
    pos = np.asarray(cluster_table)[counts - 1]          # [B, P, C]
    if not (pos == pos[0]).all():
        raise NotImplementedError("per-batch cluster tables not supported")
    p0 = pos[0]                                          # [P, C]

    scale = 1.0 / np.sqrt(np.float32(DEPTH))
    s_idx = p0.ravel()
    c_idx = np.repeat(np.arange(P), C)

    def build_table(W, sc):
        A = np.zeros((H, S + 1, P), np.float32)
        np.add.at(A, (np.arange(H)[:, None], s_idx[None, :], c_idx[None, :]),
                  np.asarray(W, np.float32).reshape(H, P * C) * sc)
        return np.ascontiguousarray(A[:, :S, :])

    AE = build_table(EW, scale)
    AF = build_table(FW, 1.0)
    # pack adjacent heads side by side: [NPAIR, S, 128]
    AE = np.ascontiguousarray(
        AE.reshape(NPAIR, 2, S, P).transpose(0, 2, 1, 3).reshape(NPAIR, S, 128))
    AF = np.ascontiguousarray(
        AF.reshape(NPAIR, 2, S, P).transpose(0, 2, 1, 3).reshape(NPAIR, S, 128))

    # conv -> 5 tap matrices
    wp = np.arange(P)[:, None]
    jj = np.arange(P)[None, :]
    ii = wp - jj + 31
    valid = (ii >= 0) & (ii < P)
    ii = np.clip(ii, 0, P - 1)
    M = {t: np.zeros((P, P), np.float32) for t in range(-2, 3)}
    for cw, hk in ((conv_w1, 1), (conv_w3, 3), (conv_w5, 5)):
        cw = np.asarray(cw, np.float32)
        pad = (hk - 1) // 2
        for dy in range(hk):
            filt = cw[dy, :, 0, 0]
            M[dy - pad] += np.where(valid, filt[ii], 0.0) / 3.0
    BDM = np.zeros((5, 128, 128), np.float32)
    for ti in range(5):
        BDM[ti, :64, :64] = M[ti - 2]
        BDM[ti, 64:, 64:] = M[ti - 2]
    bbar = float(np.asarray(conv_b, np.float32).mean())
    if abs(bbar) > 1e-30:
        raise NotImplementedError("nonzero conv bias not folded")

    ones_bd = np.zeros((128, 128), np.float32)
    ones_bd[:64, :64] = 1.0
    ones_bd[64:, 64:] = 1.0

    # shard + transpose x
    xsh = x.reshape(NCORES, BLOC, S, D)
    in_maps = []
    shared = dict(
        wq=np.asarray(wq, np.float32).astype(bf),
        wk=np.asarray(wk, np.float32).astype(bf),
        wv=np.asarray(wv, np.float32).astype(bf),
        dw=np.asarray(dense_w, np.float32).astype(bf),
        dbb=np.asarray(dense_b, np.float32).reshape(1, -1).astype(bf),
        ae=AE.astype(bf), af=AF.astype(bf), bdm=BDM.astype(bf),
        onesbd=ones_bd.astype(bf),
    )
    for c in range(NCORES):
        m = dict(shared)
        m["xT"] = np.ascontiguousarray(xsh[c].transpose(0, 2, 1)).astype(bf)
        in_maps.append(m)
    return in_maps


def _run(in_maps, trace=False, tmpdir=None):
    from concourse.bass_utils import run_bass_kernel_spmd
    if "nc" not in _CACHE:
        _CACHE["nc"] = _build_nc()
    kw = {}
    if trace:
        _install_ntff_hook()
        kw = dict(trace=True, tmpdir=tmpdir)
    return run_bass_kernel_spmd(_CACHE["nc"], in_maps,
                                core_ids=list(range(NCORES)), **kw)


def _install_ntff_hook():
    import types, importlib.util as ilu
    if "antenv.axon_hooks" in sys.modules:
        return
    spec = ilu.spec_from_file_location(
        "trn_boot_mod", "/root/.axon_site/trn_agent_boot/trn_boot.py")
    tb = ilu.module_from_spec(spec)
    spec.loader.exec_module(tb)
    hook = tb._ntff_profile_via_ctypes("/opt/axon/libaxon_pjrt.so")
    mod = types.ModuleType("antenv.axon_hooks")
    mod.get_axon_ntff_profile_hook = lambda: hook
    import antenv  # noqa: F401
    sys.modules["antenv.axon_hooks"] = mod


def kernel(**inputs) -> np.ndarray:
    in_maps = _prep_inputs(**inputs)
    r = _run(in_maps)
    return np.concatenate([r.results[c]["out"] for c in range(NCORES)], axis=0)



# revision 7
# speedup vs baseline: 1.3816x; 1.3816x over previous
"""Clustered Linformer Attention — Trainium2 Bass kernel, 8 NeuronCores.

Strategy: data-parallel over batch (2 batches/core, no collectives).
Math restructuring (validated vs reference in errsim.py):
  - mask is all-ones => the gather+einsum projections become
    k_proj_h = AE_h^T X wk_h with a host-built sparse AE in [S, P] (score
    scale folded in); computed as (X^T AE) then a small head-mix with wk,
    skipping the full K/V materialization entirely.  Same for V via AF/wv.
  - the 3-kernel conv fusion over scores collapses to 5 "tap" matrices
    M_t in [P, P]:  scores_conv[s] = sum_t q[s+t] @ (k_proj^T M_t).
  - adjacent heads are packed block-diagonally so every matmul contracts
    over the full 128 partitions.
  - f16 replaces bf16 (8x lower base error), buying error budget for
    fp8e4m3 DoubleRow matmuls (2x PE throughput) on the score-only path:
    the q projection (contraction pairs over D) and the tap matmuls
    (tap pairs as DoubleRow planes; pairs (-2,-1),(0,1),(2,zero)).  kt is
    pre-scaled by 16 to stay clear of fp8 subnormals; the exp activation
    applies scale 1/16.
  - softmax Z = sum_c exp via an all-ones block-diag matmul broadcasting
    Z to all partitions; normalization is reciprocal + multiply.
  - final dense computed transposed (features on partitions) so dense_b
    folds into the scalar-engine PSUM->SBUF copy as a per-partition bias;
    output is f16 [D, S] per batch, transposed/cast to f32 on host.
"""
import sys
import numpy as np
import ml_dtypes

sys.path.insert(0, '/opt/trn_rl_repo')

B, S, D = 16, 2048, 512
H, P, C = 8, 64, 32
DEPTH = D // H           # 64
NCORES = 8
BLOC = B // NCORES       # 2 batches per core
NPAIR = H // 2           # 4 head pairs
SCH = 4                  # s-chunks of 512
SCW = S // SCH           # 512
NJ = S // 128            # 16 s-tiles of 128
NDC = D // 128           # 4 contraction chunks
SQ = S + 6               # padded q columns (2 front, 4 back)
KT_SCALE = 16.0

_CACHE = {}


def _build_nc():
    import concourse.tile as tile
    from concourse import mybir, bacc
    from concourse.ap import AP

    f32 = mybir.dt.float32
    f16 = mybir.dt.float16
    f8 = mybir.dt.float8e4
    DR = mybir.MatmulPerfMode.DoubleRow

    nc = bacc.Bacc()
    xT8 = nc.declare_dram_parameter("xT8", [BLOC, D, S], f8, isOutput=False)
    x16 = nc.declare_dram_parameter("x16", [BLOC, S, D], f16, isOutput=False)
    wq8 = nc.declare_dram_parameter("wq8", [D, D], f8, isOutput=False)
    wk = nc.declare_dram_parameter("wk", [D, D], f16, isOutput=False)
    wv = nc.declare_dram_parameter("wv", [D, D], f16, isOutput=False)
    dw = nc.declare_dram_parameter("dw", [D, D], f16, isOutput=False)
    dbb = nc.declare_dram_parameter("dbb", [NDC, 128], f32, isOutput=False)
    ae = nc.declare_dram_parameter("ae", [S, D], f16, isOutput=False)
    af = nc.declare_dram_parameter("af", [S, D], f16, isOutput=False)
    bdm = nc.declare_dram_parameter("bdm", [5, 128, 128], f16, isOutput=False)
    onesbd = nc.declare_dram_parameter("onesbd", [128, 128], f16, isOutput=False)
    out = nc.declare_dram_parameter("out", [BLOC, D, S], f16, isOutput=True)

    with tile.TileContext(nc) as tc:
        with tc.tile_pool(name="const", bufs=1) as cpool, \
             tc.tile_pool(name="big", bufs=1) as bigp, \
             tc.tile_pool(name="sm", bufs=4) as smp, \
             tc.tile_pool(name="ob", bufs=3) as obp, \
             tc.tile_pool(name="psB", bufs=5, space="PSUM") as psB, \
             tc.tile_pool(name="psS", bufs=2, space="PSUM") as psS:

            # ---- constants in SBUF ----
            wq_sb = cpool.tile([128, NDC, D], f8)
            nc.sync.dma_start(out=wq_sb, in_=wq8[:].rearrange("(o p) m -> p o m", p=128))
            ae_sb = cpool.tile([128, NJ, D], f16)
            af_sb = cpool.tile([128, NJ, D], f16)
            nc.sync.dma_start(out=ae_sb, in_=ae[:].rearrange("(j p) c -> p j c", p=128))
            nc.sync.dma_start(out=af_sb, in_=af[:].rearrange("(j p) c -> p j c", p=128))
            wk_sb = cpool.tile([128, NDC, D], f16)
            wv_sb = cpool.tile([128, NDC, D], f16)
            dw_sb = cpool.tile([128, NDC, D], f16)
            for t_sb, t_dr in ((wk_sb, wk), (wv_sb, wv), (dw_sb, dw)):
                nc.sync.dma_start(out=t_sb, in_=t_dr[:].rearrange("(o p) m -> p o m", p=128))
            bdm_sb = cpool.tile([128, 5, 128], f16)
            nc.sync.dma_start(out=bdm_sb, in_=bdm[:].rearrange("t p m -> p t m"))
            ones_sb = cpool.tile([128, 128], f16)
            nc.sync.dma_start(out=ones_sb, in_=onesbd[:])
            dbf = cpool.tile([128, NDC], f32)
            nc.sync.dma_start(out=dbf, in_=dbb[:].rearrange("o p -> p o"))

            # Per-batch state; stages are closures so the two batches can be
            # interleaved in engine program order.
            st = [dict() for _ in range(BLOC)]

            def emit_x_load(b):
                s = st[b]
                s["xt8"] = bigp.tile([128, NDC, S], f8, tag="xt8", bufs=2,
                                     name=f"xt8_{b}")
                nc.gpsimd.dma_start(
                    out=s["xt8"],
                    in_=xT8[b].rearrange("(o p) m -> p o m", p=128))
                s["x16"] = bigp.tile([128, NJ, D], f16, tag="x16", bufs=2,
                                     name=f"x16_{b}")
                nc.gpsimd.dma_start(
                    out=s["x16"],
                    in_=x16[b].rearrange("(j p) d -> p j d", p=128))

            def emit_qt(b, pr, n):
                # qT chunk via fp8 DoubleRow: contraction pairs over dc.
                s = st[b]
                if pr == 0 and n == 0:
                    s["q8"] = bigp.tile([128, NPAIR, SQ], f8, tag="q8",
                                        bufs=2, name=f"q8_{b}")
                    nc.vector.memset(s["q8"][:, :, 0:2], 0.0)
                    nc.vector.memset(s["q8"][:, :, 2 + S:], 0.0)
                ps_q = psB.tile([128, SCW], f32, tag="ps512")
                for i in range(2):
                    nc.tensor.matmul(
                        ps_q,
                        wq_sb[:, 2 * i:2 * i + 2, 128 * pr:128 * (pr + 1)],
                        s["xt8"][:, 2 * i:2 * i + 2, SCW * n:SCW * (n + 1)],
                        start=(i == 0), stop=(i == 1), perf_mode=DR)
                nc.scalar.copy(
                    out=s["q8"][:, pr, 2 + SCW * n:2 + SCW * (n + 1)],
                    in_=ps_q)

            def emit_sweep(b, which, dblk):
                # kp_preT[:, dblk, :] = sum_j x16[:, j, dblk]^T @ A[:, j, :]
                s = st[b]
                key = "kpET" if which == "E" else "kpFT"
                if dblk == 0:
                    s[key] = bigp.tile([128, NDC, D], f16, tag=key, bufs=2,
                                       name=f"{key}_{b}")
                a_sb = ae_sb if which == "E" else af_sb
                ps = psB.tile([128, D], f32, tag="ps512")
                for j in range(NJ):
                    nc.tensor.matmul(
                        ps,
                        s["x16"][:, j, 128 * dblk:128 * (dblk + 1)],
                        a_sb[:, j, :],
                        start=(j == 0), stop=(j == NJ - 1))
                nc.vector.tensor_copy(out=s[key][:, dblk, :], in_=ps)

            def emit_mix(b, pr):
                # kp/vp diag blocks: kp = (X^T AE)_pr^T @ wk_pr (contract D)
                s = st[b]
                if pr == 0:
                    s["kp"] = bigp.tile([128, NPAIR, 128], f16, tag="kpbd",
                                        bufs=2, name=f"kp_{b}")
                    s["vp"] = bigp.tile([128, NPAIR, 128], f16, tag="vpbd",
                                        bufs=2, name=f"vp_{b}")
                    nc.vector.memset(s["kp"], 0.0)
                    nc.vector.memset(s["vp"], 0.0)
                cols = slice(128 * pr, 128 * (pr + 1))
                for src, w_sb, dstk in ((s["kpET"], wk_sb, "kp"),
                                        (s["kpFT"], wv_sb, "vp")):
                    ps_p = psS.tile([128, 128], f32, tag="pssmall")
                    for dc in range(NDC):
                        nc.tensor.matmul(
                            ps_p, src[:, dc, cols], w_sb[:, dc, cols],
                            start=(dc == 0), stop=(dc == NDC - 1))
                    dst = s[dstk]
                    nc.vector.tensor_copy(
                        out=dst[0:64, pr, 0:64], in_=ps_p[0:64, 0:64])
                    nc.vector.tensor_copy(
                        out=dst[64:128, pr, 64:128], in_=ps_p[64:128, 64:128])

            def emit_kt(b, pr):
                # kt8[:, pr, pi, pl, :] = fp8(KT_SCALE * (kp_pr^T-contracted
                # against M_t)); taps as DoubleRow pairs (-2,-1),(0,1),(2,0).
                s = st[b]
                if pr == 0:
                    s["kt8"] = bigp.tile([128, NPAIR, 3, 2, 128], f8,
                                         tag="kt8", bufs=2, name=f"kt8_{b}")
                    nc.vector.memset(s["kt8"][:, :, 2, 1, :], 0.0)
                    s["concat"] = bigp.tile([128, NPAIR, S], f16,
                                            tag="concatT", bufs=2,
                                            name=f"concat_{b}")
                for t in range(5):
                    ps_b = psS.tile([128, 128], f32, tag="pssmall")
                    nc.tensor.matmul(ps_b, s["kp"][:, pr, :], bdm_sb[:, t, :],
                                     start=True, stop=True)
                    nc.vector.tensor_scalar_mul(
                        s["kt8"][:, pr, t // 2, t % 2, :], ps_b, KT_SCALE)

            def _q_pair_ap(s, pr, n, pi):
                # rhs [128, 2, 512]: plane 0 = taps 2*pi-2, plane 1 = +1,
                # built as an overlapping strided AP (plane stride = 1 col).
                base = s["q8"][:, pr, SCW * n + 2 * pi: SCW * n + 2 * pi + 512]
                ap = [list(p) for p in base.ap]
                assert len(ap) == 2 and ap[1][0] == 1
                return AP(tensor=base.tensor, offset=base.offset,
                          ap=[ap[0], [1, 2], [1, 512]])

            def emit_att(b, pr, n):
                s = st[b]
                ps_sc = psB.tile([128, SCW], f32, tag="ps512")
                for pi in range(3):
                    nc.tensor.matmul(
                        ps_sc,
                        s["kt8"][:, pr, pi, :, :],
                        _q_pair_ap(s, pr, n, pi),
                        start=(pi == 0), stop=(pi == 2), perf_mode=DR)
                expt = smp.tile([128, SCW], f16, tag="expt")
                nc.scalar.activation(
                    out=expt, in_=ps_sc,
                    func=mybir.ActivationFunctionType.Exp,
                    scale=1.0 / KT_SCALE)
                ps_z = psB.tile([128, SCW], f32, tag="ps512")
                nc.tensor.matmul(ps_z, ones_sb, expt, start=True, stop=True)
                ps_at = psB.tile([128, SCW], f32, tag="ps512")
                nc.tensor.matmul(ps_at, s["vp"][:, pr, :], expt,
                                 start=True, stop=True)
                # 1/Z: approx reciprocal (~18 bits, single DVE op).
                rzb = smp.tile([128, SCW], f32, tag="rzb")
                nc.vector.reciprocal_approx_fast(out=rzb, in_=ps_z)
                nc.vector.tensor_mul(
                    out=s["concat"][:, pr, SCW * n:SCW * (n + 1)],
                    in0=ps_at, in1=rzb)

            def emit_dense(b, dblk, n):
                s = st[b]
                ps_d = psB.tile([128, SCW], f32, tag="ps512")
                cols = slice(128 * dblk, 128 * (dblk + 1))
                for dc in range(NDC):
                    nc.tensor.matmul(
                        ps_d,
                        dw_sb[:, dc, cols],
                        s["concat"][:, dc, SCW * n:SCW * (n + 1)],
                        start=(dc == 0), stop=(dc == NDC - 1))
                obuf = obp.tile([128, SCW], f16, tag="obuf")
                nc.scalar.activation(
                    out=obuf, in_=ps_d,
                    func=mybir.ActivationFunctionType.Identity,
                    bias=dbf[:, dblk:dblk + 1])
                nc.sync.dma_start(
                    out=out[b, cols, SCW * n:SCW * (n + 1)], in_=obuf)

            # ---- emission schedule ----
            emit_x_load(0)
            emit_x_load(1)
            for pr in range(NPAIR):
                for n in range(SCH):
                    emit_qt(0, pr, n)
            for dblk in range(NDC):
                emit_sweep(0, "E", dblk)
            for dblk in range(NDC):
                emit_sweep(0, "F", dblk)
            for pr in range(NPAIR):
                emit_mix(0, pr)
            for pr in range(NPAIR):
                emit_kt(0, pr)
            # batch-0 attention interleaved with batch-1 qT + sweeps + mix
            fillers = [(emit_qt, (1, pr, n)) for pr in range(NPAIR)
                       for n in range(SCH)] + \
                      [(emit_sweep, (1, "E", d)) for d in range(NDC)] + \
                      [(emit_sweep, (1, "F", d)) for d in range(NDC)] + \
                      [(emit_mix, (1, pr)) for pr in range(NPAIR)] + \
                      [(emit_kt, (1, pr)) for pr in range(NPAIR)]
            fi = 0
            for n in range(SCH):
                for pr in range(NPAIR):
                    emit_att(0, pr, n)
                    for _ in range(2):
                        if fi < len(fillers):
                            f, a = fillers[fi]; f(*a); fi += 1
            while fi < len(fillers):
                f, a = fillers[fi]; f(*a); fi += 1
            # batch-1 attention (n-outer) interleaved with batch-0 dense;
            # once chunk n is done for all pairs, batch-1 dense joins too.
            d0 = [(dblk, n) for n in range(SCH) for dblk in range(NDC)]
            d0i = 0
            for n in range(SCH):
                for pr in range(NPAIR):
                    emit_att(1, pr, n)
                    if d0i < len(d0):
                        emit_dense(0, *d0[d0i]); d0i += 1
                if n > 0:
                    for dblk in range(NDC):
                        emit_dense(1, dblk, n - 1)
            while d0i < len(d0):
                emit_dense(0, *d0[d0i]); d0i += 1
            for dblk in range(NDC):
                emit_dense(1, dblk, SCH - 1)

    nc.finalize()
    return nc


def _prep_inputs(x, mask, wq, wk, wv, EW, FW, conv_w1, conv_w3, conv_w5, conv_b,
                 dense_w, dense_b, cluster_table):
    """Host-side restructuring -> per-core input maps."""
    f8 = ml_dtypes.float8_e4m3
    x = np.ascontiguousarray(np.asarray(x, np.float32))
    mask = np.asarray(mask)
    counts = np.clip(mask.astype(np.int64).sum(1), 1, S)
    pos = np.asarray(cluster_table)[counts - 1]          # [B, P, C]
    if not (pos == pos[0]).all():
        raise NotImplementedError("per-batch cluster tables not supported")
    p0 = pos[0]                                          # [P, C]

    scale = 1.0 / np.sqrt(np.float32(DEPTH))
    s_idx = p0.ravel()
    c_idx = np.repeat(np.arange(P), C)

    def build_table(W, sc):
        A = np.zeros((H, S + 1, P), np.float32)
        np.add.at(A, (np.arange(H)[:, None], s_idx[None, :], c_idx[None, :]),
                  np.asarray(W, np.float32).reshape(H, P * C) * sc)
        return A[:, :S, :]

    AE = build_table(EW, scale)                          # [H, S, P]
    AF = build_table(FW, 1.0)
    # cols ordered (head, cluster) so pair pr occupies cols 128pr..128pr+128
    AE = np.ascontiguousarray(AE.transpose(1, 0, 2).reshape(S, D))
    AF = np.ascontiguousarray(AF.transpose(1, 0, 2).reshape(S, D))

    # conv -> 5 tap matrices
    wp = np.arange(P)[:, None]
    jj = np.arange(P)[None, :]
    ii = wp - jj + 31
    valid = (ii >= 0) & (ii < P)
    ii = np.clip(ii, 0, P - 1)
    M = {t: np.zeros((P, P), np.float32) for t in range(-2, 3)}
    for cw, hk in ((conv_w1, 1), (conv_w3, 3), (conv_w5, 5)):
        cw = np.asarray(cw, np.float32)
        pad = (hk - 1) // 2
        for dy in range(hk):
            filt = cw[dy, :, 0, 0]
            M[dy - pad] += np.where(valid, filt[ii], 0.0) / 3.0
    BDM = np.zeros((5, 128, 128), np.float32)
    for ti in range(5):
        BDM[ti, :64, :64] = M[ti - 2]
        BDM[ti, 64:, 64:] = M[ti - 2]
    bbar = float(np.asarray(conv_b, np.float32).mean())
    if abs(bbar) > 1e-30:
        raise NotImplementedError("nonzero conv bias not folded")

    ones_bd = np.zeros((128, 128), np.float32)
    ones_bd[:64, :64] = 1.0
    ones_bd[64:, 64:] = 1.0

    xsh = x.reshape(NCORES, BLOC, S, D)
    in_maps = []
    shared = dict(
        wq8=np.asarray(wq, np.float32).astype(f8),
        wk=np.asarray(wk, np.float32).astype(np.float16),
        wv=np.asarray(wv, np.float32).astype(np.float16),
        dw=np.asarray(dense_w, np.float32).astype(np.float16),
        dbb=np.ascontiguousarray(
            np.asarray(dense_b, np.float32).reshape(NDC, 128)),
        ae=AE.astype(np.float16), af=AF.astype(np.float16),
        bdm=BDM.astype(np.float16),
        onesbd=ones_bd.astype(np.float16),
    )
    for c in range(NCORES):
        m = dict(shared)
        m["xT8"] = np.ascontiguousarray(xsh[c].transpose(0, 2, 1)).astype(f8)
        m["x16"] = xsh[c].astype(np.float16)
        in_maps.append(m)
    return in_maps


def _run(in_maps, trace=False, tmpdir=None):
    from concourse.bass_utils import run_bass_kernel_spmd
    if "nc" not in _CACHE:
        _CACHE["nc"] = _build_nc()
    kw = {}
    if trace:
        _install_ntff_hook()
        kw = dict(trace=True, tmpdir=tmpdir)
    return run_bass_kernel_spmd(_CACHE["nc"], in_maps,
                                core_ids=list(range(NCORES)), **kw)


def _install_ntff_hook():
    import types, importlib.util as ilu
    if "antenv.axon_hooks" in sys.modules:
        return
    spec = ilu.spec_from_file_location(
        "trn_boot_mod", "/root/.axon_site/trn_agent_boot/trn_boot.py")
    tb = ilu.module_from_spec(spec)
    spec.loader.exec_module(tb)
    hook = tb._ntff_profile_via_ctypes("/opt/axon/libaxon_pjrt.so")
    mod = types.ModuleType("antenv.axon_hooks")
    mod.get_axon_ntff_profile_hook = lambda: hook
    import antenv  # noqa: F401
    sys.modules["antenv.axon_hooks"] = mod


def kernel(**inputs) -> np.ndarray:
    in_maps = _prep_inputs(**inputs)
    r = _run(in_maps)
    outs = [np.ascontiguousarray(
        r.results[c]["out"].transpose(0, 2, 1)).astype(np.float32)
        for c in range(NCORES)]
    return np.concatenate(outs, axis=0)


# revision 13
# speedup vs baseline: 1.4788x; 1.0703x over previous
"""Clustered Linformer Attention — Trainium2 Bass kernel, 8 NeuronCores.

Strategy: data-parallel over batch (2 batches/core, no collectives).
Math restructuring (validated vs reference in errsim.py):
  - mask is all-ones => the gather+einsum projections become
    k_proj_h = AE_h^T X wk_h with a host-built sparse AE in [S, P] (score
    scale folded in); computed as (X^T AE) then a small head-mix with wk,
    skipping the full K/V materialization entirely.  Same for V via AF/wv.
  - the 3-kernel conv fusion over scores collapses to 5 "tap" matrices
    M_t in [P, P]:  scores_conv[s] = sum_t q[s+t] @ (k_proj^T M_t).
  - adjacent heads are packed block-diagonally so every matmul contracts
    over the full 128 partitions.
  - f16 replaces bf16 (8x lower base error), buying error budget for
    fp8e4m3 DoubleRow matmuls (2x PE throughput) on the score-only path:
    the q projection (contraction pairs over D) and the tap matmuls
    (tap pairs as DoubleRow planes; pairs (-2,-1),(0,1),(2,zero)).  kt is
    pre-scaled by 16 to stay clear of fp8 subnormals; the exp activation
    applies scale 1/16.
  - softmax Z = sum_c exp via an all-ones block-diag matmul broadcasting
    Z to all partitions; normalization is reciprocal + multiply.
  - final dense computed transposed (features on partitions) so dense_b
    folds into the scalar-engine PSUM->SBUF copy as a per-partition bias;
    output is f16 [D, S] per batch, transposed/cast to f32 on host.
"""
import sys
import numpy as np
import ml_dtypes

sys.path.insert(0, '/opt/trn_rl_repo')

B, S, D = 16, 2048, 512
H, P, C = 8, 64, 32
DEPTH = D // H           # 64
NCORES = 8
BLOC = B // NCORES       # 2 batches per core
NPAIR = H // 2           # 4 head pairs
SCH = 4                  # s-chunks of 512
SCW = S // SCH           # 512
NJ = S // 128            # 16 s-tiles of 128
NDC = D // 128           # 4 contraction chunks
SQ = S + 6               # padded q columns (2 front, 4 back)
KT_SCALE = 16.0

_CACHE = {}


def _build_nc():
    import concourse.tile as tile
    from concourse import mybir, bacc
    from concourse.ap import AP

    f32 = mybir.dt.float32
    f16 = mybir.dt.float16
    f8 = mybir.dt.float8e4
    DR = mybir.MatmulPerfMode.DoubleRow

    nc = bacc.Bacc()
    xT8 = nc.declare_dram_parameter("xT8", [BLOC, D, S], f8, isOutput=False)
    x16 = nc.declare_dram_parameter("x16", [BLOC, S, D], f16, isOutput=False)
    wq8 = nc.declare_dram_parameter("wq8", [D, D], f8, isOutput=False)
    wk = nc.declare_dram_parameter("wk", [D, D], f16, isOutput=False)
    wv = nc.declare_dram_parameter("wv", [D, D], f16, isOutput=False)
    dw = nc.declare_dram_parameter("dw", [D, D], f16, isOutput=False)
    dbb = nc.declare_dram_parameter("dbb", [NDC, 128], f32, isOutput=False)
    ae = nc.declare_dram_parameter("ae", [S, D], f16, isOutput=False)
    af = nc.declare_dram_parameter("af", [S, D], f16, isOutput=False)
    bdm = nc.declare_dram_parameter("bdm", [5, 128, 128], f16, isOutput=False)
    onesbd = nc.declare_dram_parameter("onesbd", [128, 128], f16, isOutput=False)
    out = nc.declare_dram_parameter("out", [BLOC, D, S], f16, isOutput=True)

    with tile.TileContext(nc) as tc:
        with tc.tile_pool(name="const", bufs=1) as cpool, \
             tc.tile_pool(name="big", bufs=1) as bigp, \
             tc.tile_pool(name="sm", bufs=4) as smp, \
             tc.tile_pool(name="ob", bufs=3) as obp, \
             tc.tile_pool(name="psB", bufs=6, space="PSUM") as psB, \
             tc.tile_pool(name="psS", bufs=2, space="PSUM") as psS:

            # ---- constants in SBUF (spread across DMA queues) ----
            wq_sb = cpool.tile([128, NDC, D], f8)
            nc.sync.dma_start(out=wq_sb, in_=wq8[:].rearrange("(o p) m -> p o m", p=128))
            ae_sb = cpool.tile([128, NJ, D], f16)
            af_sb = cpool.tile([128, NJ, D], f16)
            nc.sync.dma_start(out=ae_sb, in_=ae[:].rearrange("(j p) c -> p j c", p=128))
            nc.scalar.dma_start(out=af_sb, in_=af[:].rearrange("(j p) c -> p j c", p=128))
            wk_sb = cpool.tile([128, NDC, D], f16)
            wv_sb = cpool.tile([128, NDC, D], f16)
            dw_sb = cpool.tile([128, NDC, D], f16)
            nc.scalar.dma_start(out=wk_sb, in_=wk[:].rearrange("(o p) m -> p o m", p=128))
            nc.sync.dma_start(out=wv_sb, in_=wv[:].rearrange("(o p) m -> p o m", p=128))
            nc.scalar.dma_start(out=dw_sb, in_=dw[:].rearrange("(o p) m -> p o m", p=128))
            bdm_sb = cpool.tile([128, 5, 128], f16)
            nc.sync.dma_start(out=bdm_sb, in_=bdm[:].rearrange("t p m -> p t m"))
            ones_sb = cpool.tile([128, 128], f16)
            nc.sync.dma_start(out=ones_sb, in_=onesbd[:])
            dbf = cpool.tile([128, NDC], f32)
            nc.sync.dma_start(out=dbf, in_=dbb[:].rearrange("o p -> p o"))

            # Per-batch state; stages are closures so the two batches can be
            # interleaved in engine program order.
            st = [dict() for _ in range(BLOC)]

            def emit_x_load(b):
                # separate DMA queue per batch so both load in parallel
                s = st[b]
                eng = nc.gpsimd if b == 0 else nc.scalar
                s["xt8"] = bigp.tile([128, NDC, S], f8, tag="xt8", bufs=2,
                                     name=f"xt8_{b}")
                eng.dma_start(
                    out=s["xt8"],
                    in_=xT8[b].rearrange("(o p) m -> p o m", p=128))
                s["x16"] = bigp.tile([128, NJ, D], f16, tag="x16", bufs=2,
                                     name=f"x16_{b}")
                eng.dma_start(
                    out=s["x16"],
                    in_=x16[b].rearrange("(j p) d -> p j d", p=128))

            def emit_qt(b, pr, n):
                # qT chunk via fp8 DoubleRow: contraction pairs over dc.
                s = st[b]
                if pr == 0 and n == 0:
                    s["q8"] = bigp.tile([128, NPAIR, SQ], f8, tag="q8",
                                        bufs=2, name=f"q8_{b}")
                    nc.vector.memset(s["q8"][:, :, 0:2], 0.0)
                    nc.vector.memset(s["q8"][:, :, 2 + S:], 0.0)
                ps_q = psB.tile([128, SCW], f32, tag="ps512")
                for i in range(2):
                    nc.tensor.matmul(
                        ps_q,
                        wq_sb[:, 2 * i:2 * i + 2, 128 * pr:128 * (pr + 1)],
                        s["xt8"][:, 2 * i:2 * i + 2, SCW * n:SCW * (n + 1)],
                        start=(i == 0), stop=(i == 1), perf_mode=DR)
                nc.scalar.copy(
                    out=s["q8"][:, pr, 2 + SCW * n:2 + SCW * (n + 1)],
                    in_=ps_q)

            def emit_sweep(b, which, dblk):
                # kp_preT[:, dblk, :] = sum_j x16[:, j, dblk]^T @ A[:, j, :]
                s = st[b]
                key = "kpET" if which == "E" else "kpFT"
                if dblk == 0:
                    s[key] = bigp.tile([128, NDC, D], f16, tag=key, bufs=2,
                                       name=f"{key}_{b}")
                a_sb = ae_sb if which == "E" else af_sb
                ps = psB.tile([128, D], f32, tag="ps512")
                for j in range(NJ):
                    nc.tensor.matmul(
                        ps,
                        s["x16"][:, j, 128 * dblk:128 * (dblk + 1)],
                        a_sb[:, j, :],
                        start=(j == 0), stop=(j == NJ - 1))
                nc.vector.tensor_copy(out=s[key][:, dblk, :], in_=ps)

            def emit_mix(b, pr):
                # kp/vp diag blocks: kp = (X^T AE)_pr^T @ wk_pr (contract D)
                s = st[b]
                if pr == 0:
                    s["kp"] = bigp.tile([128, NPAIR, 128], f16, tag="kpbd",
                                        bufs=2, name=f"kp_{b}")
                    s["vp"] = bigp.tile([128, NPAIR, 128], f16, tag="vpbd",
                                        bufs=2, name=f"vp_{b}")
                    nc.vector.memset(s["kp"], 0.0)
                    nc.vector.memset(s["vp"], 0.0)
                cols = slice(128 * pr, 128 * (pr + 1))
                for src, w_sb, dstk in ((s["kpET"], wk_sb, "kp"),
                                        (s["kpFT"], wv_sb, "vp")):
                    ps_p = psS.tile([128, 128], f32, tag="pssmall")
                    for dc in range(NDC):
                        nc.tensor.matmul(
                            ps_p, src[:, dc, cols], w_sb[:, dc, cols],
                            start=(dc == 0), stop=(dc == NDC - 1))
                    dst = s[dstk]
                    nc.vector.tensor_copy(
                        out=dst[0:64, pr, 0:64], in_=ps_p[0:64, 0:64])
                    nc.vector.tensor_copy(
                        out=dst[64:128, pr, 64:128], in_=ps_p[64:128, 64:128])

            def emit_kt(b, pr):
                # kt8[:, pr, pi, pl, :] = fp8(KT_SCALE * (kp_pr^T-contracted
                # against M_t)); taps as DoubleRow pairs (-2,-1),(0,1),(2,0).
                s = st[b]
                if pr == 0:
                    s["kt8"] = bigp.tile([128, NPAIR, 3, 2, 128], f8,
                                         tag="kt8", bufs=2, name=f"kt8_{b}")
                    nc.vector.memset(s["kt8"][:, :, 2, 1, :], 0.0)
                    s["concat"] = bigp.tile([128, NPAIR, S], f16,
                                            tag="concatT", bufs=2,
                                            name=f"concat_{b}")
                for t in range(5):
                    ps_b = psS.tile([128, 128], f32, tag="pssmall")
                    nc.tensor.matmul(ps_b, s["kp"][:, pr, :], bdm_sb[:, t, :],
                                     start=True, stop=True)
                    nc.vector.tensor_scalar_mul(
                        s["kt8"][:, pr, t // 2, t % 2, :], ps_b, KT_SCALE)

            def _q_pair_ap(s, pr, n, pi):
                # rhs [128, 2, 512]: plane 0 = taps 2*pi-2, plane 1 = +1,
                # built as an overlapping strided AP (plane stride = 1 col).
                base = s["q8"][:, pr, SCW * n + 2 * pi: SCW * n + 2 * pi + 512]
                ap = [list(p) for p in base.ap]
                assert len(ap) == 2 and ap[1][0] == 1
                return AP(tensor=base.tensor, offset=base.offset,
                          ap=[ap[0], [1, 2], [1, 512]])

            def emit_att_score(b, pr, n):
                # taps (3 fp8 DoubleRow matmuls) + exp on the scalar engine.
                # Z/AV are deferred to emit_att_post so the PE can run other
                # work while the exp finishes (software pipelining).
                s = st[b]
                ps_sc = psB.tile([128, SCW], f32, tag="ps512")
                for pi in range(3):
                    nc.tensor.matmul(
                        ps_sc,
                        s["kt8"][:, pr, pi, :, :],
                        _q_pair_ap(s, pr, n, pi),
                        start=(pi == 0), stop=(pi == 2), perf_mode=DR)
                expt = smp.tile([128, SCW], f16, tag="expt")
                nc.scalar.activation(
                    out=expt, in_=ps_sc,
                    func=mybir.ActivationFunctionType.Exp,
                    scale=1.0 / KT_SCALE)
                return expt

            def emit_att_post(b, pr, n, expt):
                s = st[b]
                ps_z = psB.tile([128, SCW], f32, tag="ps512")
                nc.tensor.matmul(ps_z, ones_sb, expt, start=True, stop=True)
                ps_at = psB.tile([128, SCW], f32, tag="ps512")
                nc.tensor.matmul(ps_at, s["vp"][:, pr, :], expt,
                                 start=True, stop=True)
                # 1/Z: approx reciprocal (~18 bits, single DVE op).
                rzb = smp.tile([128, SCW], f32, tag="rzb")
                nc.vector.reciprocal_approx_fast(out=rzb, in_=ps_z)
                nc.vector.tensor_mul(
                    out=s["concat"][:, pr, SCW * n:SCW * (n + 1)],
                    in0=ps_at, in1=rzb)

            def emit_dense(b, dblk, n):
                s = st[b]
                ps_d = psB.tile([128, SCW], f32, tag="ps512")
                cols = slice(128 * dblk, 128 * (dblk + 1))
                for dc in range(NDC):
                    nc.tensor.matmul(
                        ps_d,
                        dw_sb[:, dc, cols],
                        s["concat"][:, dc, SCW * n:SCW * (n + 1)],
                        start=(dc == 0), stop=(dc == NDC - 1))
                obuf = obp.tile([128, SCW], f16, tag="obuf")
                nc.scalar.activation(
                    out=obuf, in_=ps_d,
                    func=mybir.ActivationFunctionType.Identity,
                    bias=dbf[:, dblk:dblk + 1])
                nc.sync.dma_start(
                    out=out[b, cols, SCW * n:SCW * (n + 1)], in_=obuf)

            # ---- emission schedule ----
            emit_x_load(0)
            emit_x_load(1)
            for pr in range(NPAIR):
                for n in range(SCH):
                    emit_qt(0, pr, n)
            for dblk in range(NDC):
                emit_sweep(0, "E", dblk)
            for dblk in range(NDC):
                emit_sweep(0, "F", dblk)
            for pr in range(NPAIR):
                emit_mix(0, pr)
            for pr in range(NPAIR):
                emit_kt(0, pr)
            # batch-0 attention interleaved with batch-1 qT + sweeps + mix.
            # att units are software-pipelined: taps+exp of unit u issue,
            # then (filler PE work), then Z/AV of unit u-1 — so the PE never
            # stalls on the exp.
            fillers = [(emit_qt, (1, pr, n)) for pr in range(NPAIR)
                       for n in range(SCH)] + \
                      [(emit_sweep, (1, "E", d)) for d in range(NDC)] + \
                      [(emit_sweep, (1, "F", d)) for d in range(NDC)] + \
                      [(emit_mix, (1, pr)) for pr in range(NPAIR)] + \
                      [(emit_kt, (1, pr)) for pr in range(NPAIR)]
            fi = 0
            prev = None
            for n in range(SCH):
                for pr in range(NPAIR):
                    expt = emit_att_score(0, pr, n)
                    if fi < len(fillers):
                        f, a = fillers[fi]; f(*a); fi += 1
                    if prev is not None:
                        emit_att_post(0, *prev)
                    prev = (pr, n, expt)
            emit_att_post(0, *prev)
            while fi < len(fillers):
                f, a = fillers[fi]; f(*a); fi += 1
            # batch-1 attention (n-outer, pipelined) interleaved with
            # batch-0 dense; batch-1 dense joins as chunks complete.
            d0 = [(dblk, n) for n in range(SCH) for dblk in range(NDC)]
            d0i = 0
            prev = None
            done_n = -1
            for n in range(SCH):
                for pr in range(NPAIR):
                    expt = emit_att_score(1, pr, n)
                    if d0i < len(d0):
                        emit_dense(0, *d0[d0i]); d0i += 1
                    if prev is not None:
                        emit_att_post(1, *prev)
                        if prev[0] == NPAIR - 1:
                            done_n = prev[1]
                    prev = (pr, n, expt)
                    if done_n >= 0:
                        emit_dense(1, pr, done_n)
                        if pr == NPAIR - 1:
                            done_n = -1
            emit_att_post(1, *prev)
            while d0i < len(d0):
                emit_dense(0, *d0[d0i]); d0i += 1
            for dblk in range(NDC):
                emit_dense(1, dblk, SCH - 1)

    nc.finalize()
    return nc


def _prep_inputs(x, mask, wq, wk, wv, EW, FW, conv_w1, conv_w3, conv_w5, conv_b,
                 dense_w, dense_b, cluster_table):
    """Host-side restructuring -> per-core input maps."""
    f8 = ml_dtypes.float8_e4m3
    x = np.ascontiguousarray(np.asarray(x, np.float32))
    mask = np.asarray(mask)
    counts = np.clip(mask.astype(np.int64).sum(1), 1, S)
    pos = np.asarray(cluster_table)[counts - 1]          # [B, P, C]
    if not (pos == pos[0]).all():
        raise NotImplementedError("per-batch cluster tables not supported")
    p0 = pos[0]                                          # [P, C]

    scale = 1.0 / np.sqrt(np.float32(DEPTH))
    s_idx = p0.ravel()
    c_idx = np.repeat(np.arange(P), C)

    def build_table(W, sc):
        A = np.zeros((H, S + 1, P), np.float32)
        np.add.at(A, (np.arange(H)[:, None], s_idx[None, :], c_idx[None, :]),
                  np.asarray(W, np.float32).reshape(H, P * C) * sc)
        return A[:, :S, :]

    AE = build_table(EW, scale)                          # [H, S, P]
    AF = build_table(FW, 1.0)
    # cols ordered (head, cluster) so pair pr occupies cols 128pr..128pr+128
    AE = np.ascontiguousarray(AE.transpose(1, 0, 2).reshape(S, D))
    AF = np.ascontiguousarray(AF.transpose(1, 0, 2).reshape(S, D))

    # conv -> 5 tap matrices
    wp = np.arange(P)[:, None]
    jj = np.arange(P)[None, :]
    ii = wp - jj + 31
    valid = (ii >= 0) & (ii < P)
    ii = np.clip(ii, 0, P - 1)
    M = {t: np.zeros((P, P), np.float32) for t in range(-2, 3)}
    for cw, hk in ((conv_w1, 1), (conv_w3, 3), (conv_w5, 5)):
        cw = np.asarray(cw, np.float32)
        pad = (hk - 1) // 2
        for dy in range(hk):
            filt = cw[dy, :, 0, 0]
            M[dy - pad] += np.where(valid, filt[ii], 0.0) / 3.0
    BDM = np.zeros((5, 128, 128), np.float32)
    for ti in range(5):
        BDM[ti, :64, :64] = M[ti - 2]
        BDM[ti, 64:, 64:] = M[ti - 2]
    bbar = float(np.asarray(conv_b, np.float32).mean())
    if abs(bbar) > 1e-30:
        raise NotImplementedError("nonzero conv bias not folded")

    ones_bd = np.zeros((128, 128), np.float32)
    ones_bd[:64, :64] = 1.0
    ones_bd[64:, 64:] = 1.0

    xsh = x.reshape(NCORES, BLOC, S, D)
    in_maps = []
    shared = dict(
        wq8=np.asarray(wq, np.float32).astype(f8),
        wk=np.asarray(wk, np.float32).astype(np.float16),
        wv=np.asarray(wv, np.float32).astype(np.float16),
        dw=np.asarray(dense_w, np.float32).astype(np.float16),
        dbb=np.ascontiguousarray(
            np.asarray(dense_b, np.float32).reshape(NDC, 128)),
        ae=AE.astype(np.float16), af=AF.astype(np.float16),
        bdm=BDM.astype(np.float16),
        onesbd=ones_bd.astype(np.float16),
    )
    for c in range(NCORES):
        m = dict(shared)
        m["xT8"] = np.ascontiguousarray(xsh[c].transpose(0, 2, 1)).astype(f8)
        m["x16"] = xsh[c].astype(np.float16)
        in_maps.append(m)
    return in_maps


def _run(in_maps, trace=False, tmpdir=None):
    from concourse.bass_utils import run_bass_kernel_spmd
    if "nc" not in _CACHE:
        _CACHE["nc"] = _build_nc()
    kw = {}
    if trace:
        _install_ntff_hook()
        kw = dict(trace=True, tmpdir=tmpdir)
    return run_bass_kernel_spmd(_CACHE["nc"], in_maps,
                                core_ids=list(range(NCORES)), **kw)


def _install_ntff_hook():
    import types, importlib.util as ilu
    if "antenv.axon_hooks" in sys.modules:
        return
    spec = ilu.spec_from_file_location(
        "trn_boot_mod", "/root/.axon_site/trn_agent_boot/trn_boot.py")
    tb = ilu.module_from_spec(spec)
    spec.loader.exec_module(tb)
    hook = tb._ntff_profile_via_ctypes("/opt/axon/libaxon_pjrt.so")
    mod = types.ModuleType("antenv.axon_hooks")
    mod.get_axon_ntff_profile_hook = lambda: hook
    import antenv  # noqa: F401
    sys.modules["antenv.axon_hooks"] = mod


def kernel(**inputs) -> np.ndarray:
    in_maps = _prep_inputs(**inputs)
    r = _run(in_maps)
    outs = [np.ascontiguousarray(
        r.results[c]["out"].transpose(0, 2, 1)).astype(np.float32)
        for c in range(NCORES)]
    return np.concatenate(outs, axis=0)


# revision 18
# speedup vs baseline: 1.5330x; 1.0367x over previous
"""Clustered Linformer Attention — Trainium2 Bass kernel, 8 NeuronCores.

Strategy: data-parallel over batch (2 batches/core, no collectives).
Math restructuring (validated vs reference in errsim.py):
  - mask is all-ones => the gather+einsum projections become
    k_proj_h = AE_h^T X wk_h with a host-built sparse AE in [S, P] (score
    scale folded in); computed as (X^T AE) then a small head-mix with wk,
    skipping the full K/V materialization entirely.  Same for V via AF/wv.
  - the 3-kernel conv fusion over scores collapses to 5 "tap" matrices
    M_t in [P, P]:  scores_conv[s] = sum_t q[s+t] @ (k_proj^T M_t).
  - adjacent heads are packed block-diagonally so every matmul contracts
    over the full 128 partitions.
  - f16 replaces bf16 (8x lower base error), buying error budget for
    fp8e4m3 DoubleRow matmuls (2x PE throughput) on the score-only path:
    the q projection (contraction pairs over D) and the tap matmuls
    (tap pairs as DoubleRow planes; pairs (-2,-1),(0,1),(2,zero)).  kt is
    pre-scaled by 16 to stay clear of fp8 subnormals; the exp activation
    applies scale 1/16.
  - softmax Z = sum_c exp via an all-ones block-diag matmul broadcasting
    Z to all partitions; normalization is reciprocal + multiply.
  - final dense computed transposed (features on partitions) so dense_b
    folds into the scalar-engine PSUM->SBUF copy as a per-partition bias;
    output is f16 [D, S] per batch, transposed/cast to f32 on host.
"""
import sys
import numpy as np
import ml_dtypes

sys.path.insert(0, '/opt/trn_rl_repo')

B, S, D = 16, 2048, 512
H, P, C = 8, 64, 32
DEPTH = D // H           # 64
NCORES = 8
BLOC = B // NCORES       # 2 batches per core
NPAIR = H // 2           # 4 head pairs
SCH = 4                  # s-chunks of 512
SCW = S // SCH           # 512
NJ = S // 128            # 16 s-tiles of 128
NDC = D // 128           # 4 contraction chunks
SQ = S + 6               # padded q columns (2 front, 4 back)
KT_SCALE = 16.0

_CACHE = {}


def _build_nc():
    import concourse.tile as tile
    from concourse import mybir, bacc
    from concourse.ap import AP

    f32 = mybir.dt.float32
    f16 = mybir.dt.float16
    f8 = mybir.dt.float8e4
    DR = mybir.MatmulPerfMode.DoubleRow

    # all inputs host-pre-arranged to partition-major [128, X] so every DMA
    # is a dense linear copy with maximal descriptor size
    nc = bacc.Bacc()
    xT8 = nc.declare_dram_parameter("xT8", [BLOC, 128, NDC * S], f8, isOutput=False)
    x16 = nc.declare_dram_parameter("x16", [BLOC, 128, NJ * D], f16, isOutput=False)
    wq8 = nc.declare_dram_parameter("wq8", [128, NDC * D], f8, isOutput=False)
    wk = nc.declare_dram_parameter("wk", [128, NDC * D], f16, isOutput=False)
    wv = nc.declare_dram_parameter("wv", [128, NDC * D], f16, isOutput=False)
    dw = nc.declare_dram_parameter("dw", [128, NDC * D], f16, isOutput=False)
    dbb = nc.declare_dram_parameter("dbb", [128, NDC], f32, isOutput=False)
    ae = nc.declare_dram_parameter("ae", [128, NJ * D], f16, isOutput=False)
    af = nc.declare_dram_parameter("af", [128, NJ * D], f16, isOutput=False)
    bdm = nc.declare_dram_parameter("bdm", [128, 5 * 128], f16, isOutput=False)
    onesbd = nc.declare_dram_parameter("onesbd", [128, 128], f16, isOutput=False)
    out = nc.declare_dram_parameter("out", [BLOC, D, S], f16, isOutput=True)

    with tile.TileContext(nc) as tc:
        with tc.tile_pool(name="const", bufs=1) as cpool, \
             tc.tile_pool(name="big", bufs=1) as bigp, \
             tc.tile_pool(name="sm", bufs=4) as smp, \
             tc.tile_pool(name="ob", bufs=3) as obp, \
             tc.tile_pool(name="psB", bufs=6, space="PSUM") as psB, \
             tc.tile_pool(name="psS", bufs=2, space="PSUM") as psS:

            # ---- constants in SBUF (spread across DMA queues) ----
            wq_sb = cpool.tile([128, NDC, D], f8)
            nc.sync.dma_start(out=wq_sb, in_=wq8[:].rearrange("p (o m) -> p o m", o=NDC))
            ae_sb = cpool.tile([128, NJ, D], f16)
            af_sb = cpool.tile([128, NJ, D], f16)
            nc.sync.dma_start(out=ae_sb, in_=ae[:].rearrange("p (j c) -> p j c", j=NJ))
            nc.scalar.dma_start(out=af_sb, in_=af[:].rearrange("p (j c) -> p j c", j=NJ))
            wk_sb = cpool.tile([128, NDC, D], f16)
            wv_sb = cpool.tile([128, NDC, D], f16)
            dw_sb = cpool.tile([128, NDC, D], f16)
            nc.scalar.dma_start(out=wk_sb, in_=wk[:].rearrange("p (o m) -> p o m", o=NDC))
            nc.sync.dma_start(out=wv_sb, in_=wv[:].rearrange("p (o m) -> p o m", o=NDC))
            nc.scalar.dma_start(out=dw_sb, in_=dw[:].rearrange("p (o m) -> p o m", o=NDC))
            bdm_sb = cpool.tile([128, 5, 128], f16)
            nc.sync.dma_start(out=bdm_sb, in_=bdm[:].rearrange("p (t m) -> p t m", t=5))
            ones_sb = cpool.tile([128, 128], f16)
            nc.sync.dma_start(out=ones_sb, in_=onesbd[:])
            dbf = cpool.tile([128, NDC], f32)
            nc.sync.dma_start(out=dbf, in_=dbb[:])

            # Per-batch state; stages are closures so the two batches can be
            # interleaved in engine program order.
            st = [dict() for _ in range(BLOC)]

            def emit_x_load(b):
                s = st[b]
                s["xt8"] = bigp.tile([128, NDC, S], f8, tag="xt8", bufs=2,
                                     name=f"xt8_{b}")
                nc.gpsimd.dma_start(
                    out=s["xt8"],
                    in_=xT8[b].rearrange("p (o m) -> p o m", o=NDC))
                s["x16"] = bigp.tile([128, NJ, D], f16, tag="x16", bufs=2,
                                     name=f"x16_{b}")
                nc.gpsimd.dma_start(
                    out=s["x16"],
                    in_=x16[b].rearrange("p (j d) -> p j d", j=NJ))

            def emit_qt(b, pr, n):
                # qT chunk via fp8 DoubleRow: contraction pairs over dc.
                s = st[b]
                if pr == 0 and n == 0:
                    s["q8"] = bigp.tile([128, NPAIR, SQ], f8, tag="q8",
                                        bufs=2, name=f"q8_{b}")
                    nc.vector.memset(s["q8"][:, :, 0:2], 0.0)
                    nc.vector.memset(s["q8"][:, :, 2 + S:], 0.0)
                ps_q = psB.tile([128, SCW], f32, tag="ps512")
                for i in range(2):
                    nc.tensor.matmul(
                        ps_q,
                        wq_sb[:, 2 * i:2 * i + 2, 128 * pr:128 * (pr + 1)],
                        s["xt8"][:, 2 * i:2 * i + 2, SCW * n:SCW * (n + 1)],
                        start=(i == 0), stop=(i == 1), perf_mode=DR)
                nc.scalar.copy(
                    out=s["q8"][:, pr, 2 + SCW * n:2 + SCW * (n + 1)],
                    in_=ps_q)

            def emit_sweep(b, which, dblk):
                # kp_preT[:, dblk, :] = sum_j x16[:, j, dblk]^T @ A[:, j, :]
                s = st[b]
                key = "kpET" if which == "E" else "kpFT"
                if dblk == 0:
                    s[key] = bigp.tile([128, NDC, D], f16, tag=key, bufs=2,
                                       name=f"{key}_{b}")
                a_sb = ae_sb if which == "E" else af_sb
                ps = psB.tile([128, D], f32, tag="ps512")
                for j in range(NJ):
                    nc.tensor.matmul(
                        ps,
                        s["x16"][:, j, 128 * dblk:128 * (dblk + 1)],
                        a_sb[:, j, :],
                        start=(j == 0), stop=(j == NJ - 1))
                nc.vector.tensor_copy(out=s[key][:, dblk, :], in_=ps)

            def emit_mix(b, pr):
                # kp/vp diag blocks: kp = (X^T AE)_pr^T @ wk_pr (contract D)
                s = st[b]
                if pr == 0:
                    s["kp"] = bigp.tile([128, NPAIR, 128], f16, tag="kpbd",
                                        bufs=2, name=f"kp_{b}")
                    s["vp"] = bigp.tile([128, NPAIR, 128], f16, tag="vpbd",
                                        bufs=2, name=f"vp_{b}")
                    nc.vector.memset(s["kp"], 0.0)
                    nc.vector.memset(s["vp"], 0.0)
                cols = slice(128 * pr, 128 * (pr + 1))
                for src, w_sb, dstk in ((s["kpET"], wk_sb, "kp"),
                                        (s["kpFT"], wv_sb, "vp")):
                    ps_p = psS.tile([128, 128], f32, tag="pssmall")
                    for dc in range(NDC):
                        nc.tensor.matmul(
                            ps_p, src[:, dc, cols], w_sb[:, dc, cols],
                            start=(dc == 0), stop=(dc == NDC - 1))
                    dst = s[dstk]
                    nc.vector.tensor_copy(
                        out=dst[0:64, pr, 0:64], in_=ps_p[0:64, 0:64])
                    nc.vector.tensor_copy(
                        out=dst[64:128, pr, 64:128], in_=ps_p[64:128, 64:128])

            def emit_kt(b, pr):
                # kt8[:, pr, pi, pl, :] = fp8(KT_SCALE * (kp_pr^T-contracted
                # against M_t)); taps as DoubleRow pairs (-2,-1),(0,1),(2,0).
                s = st[b]
                if pr == 0:
                    s["kt8"] = bigp.tile([128, NPAIR, 3, 2, 128], f8,
                                         tag="kt8", bufs=2, name=f"kt8_{b}")
                    nc.vector.memset(s["kt8"][:, :, 2, 1, :], 0.0)
                    s["concat"] = bigp.tile([128, NPAIR, S], f16,
                                            tag="concatT", bufs=2,
                                            name=f"concat_{b}")
                for t in range(5):
                    ps_b = psS.tile([128, 128], f32, tag="pssmall")
                    nc.tensor.matmul(ps_b, s["kp"][:, pr, :], bdm_sb[:, t, :],
                                     start=True, stop=True)
                    nc.vector.tensor_scalar_mul(
                        s["kt8"][:, pr, t // 2, t % 2, :], ps_b, KT_SCALE)

            def _q_pair_ap(s, pr, n, pi):
                # rhs [128, 2, 512]: plane 0 = taps 2*pi-2, plane 1 = +1,
                # built as an overlapping strided AP (plane stride = 1 col).
                base = s["q8"][:, pr, SCW * n + 2 * pi: SCW * n + 2 * pi + 512]
                ap = [list(p) for p in base.ap]
                assert len(ap) == 2 and ap[1][0] == 1
                return AP(tensor=base.tensor, offset=base.offset,
                          ap=[ap[0], [1, 2], [1, 512]])

            def emit_att_score(b, pr, n):
                # taps (3 fp8 DoubleRow matmuls) + exp on the scalar engine.
                # Z/AV are deferred to emit_att_post so the PE can run other
                # work while the exp finishes (software pipelining).
                s = st[b]
                ps_sc = psB.tile([128, SCW], f32, tag="ps512")
                for pi in range(3):
                    nc.tensor.matmul(
                        ps_sc,
                        s["kt8"][:, pr, pi, :, :],
                        _q_pair_ap(s, pr, n, pi),
                        start=(pi == 0), stop=(pi == 2), perf_mode=DR)
                expt = smp.tile([128, SCW], f16, tag="expt")
                nc.scalar.activation(
                    out=expt, in_=ps_sc,
                    func=mybir.ActivationFunctionType.Exp,
                    scale=1.0 / KT_SCALE)
                return expt

            def emit_att_post(b, pr, n, expt):
                s = st[b]
                ps_z = psB.tile([128, SCW], f32, tag="ps512")
                nc.tensor.matmul(ps_z, ones_sb, expt, start=True, stop=True)
                ps_at = psB.tile([128, SCW], f32, tag="ps512")
                nc.tensor.matmul(ps_at, s["vp"][:, pr, :], expt,
                                 start=True, stop=True)
                # 1/Z: approx reciprocal (~18 bits, single DVE op).
                rzb = smp.tile([128, SCW], f32, tag="rzb")
                nc.vector.reciprocal_approx_fast(out=rzb, in_=ps_z)
                nc.vector.tensor_mul(
                    out=s["concat"][:, pr, SCW * n:SCW * (n + 1)],
                    in0=ps_at, in1=rzb)

            def emit_dense(b, dblk, n):
                s = st[b]
                ps_d = psB.tile([128, SCW], f32, tag="ps512")
                cols = slice(128 * dblk, 128 * (dblk + 1))
                for dc in range(NDC):
                    nc.tensor.matmul(
                        ps_d,
                        dw_sb[:, dc, cols],
                        s["concat"][:, dc, SCW * n:SCW * (n + 1)],
                        start=(dc == 0), stop=(dc == NDC - 1))
                obuf = obp.tile([128, SCW], f16, tag="obuf")
                nc.scalar.activation(
                    out=obuf, in_=ps_d,
                    func=mybir.ActivationFunctionType.Identity,
                    bias=dbf[:, dblk:dblk + 1])
                eng = nc.sync if (dblk + n) % 2 == 0 else nc.gpsimd
                eng.dma_start(
                    out=out[b, cols, SCW * n:SCW * (n + 1)], in_=obuf)

            # ---- emission schedule ----
            emit_x_load(0)
            emit_x_load(1)
            for pr in range(NPAIR):
                for n in range(SCH):
                    emit_qt(0, pr, n)
            for dblk in range(NDC):
                emit_sweep(0, "E", dblk)
            for dblk in range(NDC):
                emit_sweep(0, "F", dblk)
            for pr in range(NPAIR):
                emit_mix(0, pr)
            for pr in range(NPAIR):
                emit_kt(0, pr)
            # batch-0 attention interleaved with batch-1 qT + sweeps + mix.
            # att units are software-pipelined: taps+exp of unit u issue,
            # then (filler PE work), then Z/AV of unit u-1 — so the PE never
            # stalls on the exp.
            fillers = [(emit_qt, (1, pr, n)) for pr in range(NPAIR)
                       for n in range(SCH)] + \
                      [(emit_sweep, (1, "E", d)) for d in range(NDC)] + \
                      [(emit_sweep, (1, "F", d)) for d in range(NDC)] + \
                      [(emit_mix, (1, pr)) for pr in range(NPAIR)] + \
                      [(emit_kt, (1, pr)) for pr in range(NPAIR)]
            fi = 0
            prev = None
            for n in range(SCH):
                for pr in range(NPAIR):
                    expt = emit_att_score(0, pr, n)
                    if fi < len(fillers):
                        f, a = fillers[fi]; f(*a); fi += 1
                    if prev is not None:
                        emit_att_post(0, *prev)
                    prev = (pr, n, expt)
            emit_att_post(0, *prev)
            while fi < len(fillers):
                f, a = fillers[fi]; f(*a); fi += 1
            # batch-1 attention (n-outer, pipelined) interleaved with
            # batch-0 dense; batch-1 dense joins as chunks complete.
            d0 = [(dblk, n) for n in range(SCH) for dblk in range(NDC)]
            d0i = 0
            prev = None
            done_n = -1
            for n in range(SCH):
                for pr in range(NPAIR):
                    expt = emit_att_score(1, pr, n)
                    if d0i < len(d0):
                        emit_dense(0, *d0[d0i]); d0i += 1
                    if prev is not None:
                        emit_att_post(1, *prev)
                        if prev[0] == NPAIR - 1:
                            done_n = prev[1]
                    prev = (pr, n, expt)
                    if done_n >= 0:
                        emit_dense(1, pr, done_n)
                        if pr == NPAIR - 1:
                            done_n = -1
            emit_att_post(1, *prev)
            while d0i < len(d0):
                emit_dense(0, *d0[d0i]); d0i += 1
            for dblk in range(NDC):
                emit_dense(1, dblk, SCH - 1)

    nc.finalize()
    return nc


def _prep_inputs(x, mask, wq, wk, wv, EW, FW, conv_w1, conv_w3, conv_w5, conv_b,
                 dense_w, dense_b, cluster_table):
    """Host-side restructuring -> per-core input maps."""
    f8 = ml_dtypes.float8_e4m3
    x = np.ascontiguousarray(np.asarray(x, np.float32))
    mask = np.asarray(mask)
    counts = np.clip(mask.astype(np.int64).sum(1), 1, S)
    pos = np.asarray(cluster_table)[counts - 1]          # [B, P, C]
    if not (pos == pos[0]).all():
        raise NotImplementedError("per-batch cluster tables not supported")
    p0 = pos[0]                                          # [P, C]

    scale = 1.0 / np.sqrt(np.float32(DEPTH))
    s_idx = p0.ravel()
    c_idx = np.repeat(np.arange(P), C)

    def build_table(W, sc):
        A = np.zeros((H, S + 1, P), np.float32)
        np.add.at(A, (np.arange(H)[:, None], s_idx[None, :], c_idx[None, :]),
                  np.asarray(W, np.float32).reshape(H, P * C) * sc)
        return A[:, :S, :]

    AE = build_table(EW, scale)                          # [H, S, P]
    AF = build_table(FW, 1.0)
    # cols ordered (head, cluster) so pair pr occupies cols 128pr..128pr+128
    AE = np.ascontiguousarray(AE.transpose(1, 0, 2).reshape(S, D))
    AF = np.ascontiguousarray(AF.transpose(1, 0, 2).reshape(S, D))

    # conv -> 5 tap matrices
    wp = np.arange(P)[:, None]
    jj = np.arange(P)[None, :]
    ii = wp - jj + 31
    valid = (ii >= 0) & (ii < P)
    ii = np.clip(ii, 0, P - 1)
    M = {t: np.zeros((P, P), np.float32) for t in range(-2, 3)}
    for cw, hk in ((conv_w1, 1), (conv_w3, 3), (conv_w5, 5)):
        cw = np.asarray(cw, np.float32)
        pad = (hk - 1) // 2
        for dy in range(hk):
            filt = cw[dy, :, 0, 0]
            M[dy - pad] += np.where(valid, filt[ii], 0.0) / 3.0
    BDM = np.zeros((5, 128, 128), np.float32)
    for ti in range(5):
        BDM[ti, :64, :64] = M[ti - 2]
        BDM[ti, 64:, 64:] = M[ti - 2]
    bbar = float(np.asarray(conv_b, np.float32).mean())
    if abs(bbar) > 1e-30:
        raise NotImplementedError("nonzero conv bias not folded")

    ones_bd = np.zeros((128, 128), np.float32)
    ones_bd[:64, :64] = 1.0
    ones_bd[64:, 64:] = 1.0

    # pre-arrange everything to partition-major [128, X] for linear DMA
    def pm_o(w):
        # [O*128, M] -> [128, O*M]  (partition = inner row index)
        w = np.asarray(w, np.float32)
        o = w.shape[0] // 128
        return np.ascontiguousarray(
            w.reshape(o, 128, -1).transpose(1, 0, 2).reshape(128, -1))

    xsh = x.reshape(NCORES, BLOC, S, D)
    in_maps = []
    shared = dict(
        wq8=pm_o(wq).astype(f8),
        wk=pm_o(wk).astype(np.float16),
        wv=pm_o(wv).astype(np.float16),
        dw=pm_o(dense_w).astype(np.float16),
        dbb=np.ascontiguousarray(np.asarray(dense_b, np.float32)
                                 .reshape(NDC, 128).T),
        ae=pm_o(AE).astype(np.float16), af=pm_o(AF).astype(np.float16),
        bdm=np.ascontiguousarray(
            BDM.transpose(1, 0, 2).reshape(128, 5 * 128)).astype(np.float16),
        onesbd=ones_bd.astype(np.float16),
    )
    for c in range(NCORES):
        m = dict(shared)
        xT = xsh[c].transpose(0, 2, 1)           # [BLOC, D, S]
        m["xT8"] = np.ascontiguousarray(
            xT.reshape(BLOC, NDC, 128, S).transpose(0, 2, 1, 3)
            .reshape(BLOC, 128, NDC * S)).astype(f8)
        m["x16"] = np.ascontiguousarray(
            xsh[c].reshape(BLOC, NJ, 128, D).transpose(0, 2, 1, 3)
            .reshape(BLOC, 128, NJ * D)).astype(np.float16)
        in_maps.append(m)
    return in_maps


def _run(in_maps, trace=False, tmpdir=None):
    from concourse.bass_utils import run_bass_kernel_spmd
    if "nc" not in _CACHE:
        _CACHE["nc"] = _build_nc()
    kw = {}
    if trace:
        _install_ntff_hook()
        kw = dict(trace=True, tmpdir=tmpdir)
    return run_bass_kernel_spmd(_CACHE["nc"], in_maps,
                                core_ids=list(range(NCORES)), **kw)


def _install_ntff_hook():
    import types, importlib.util as ilu
    if "antenv.axon_hooks" in sys.modules:
        return
    spec = ilu.spec_from_file_location(
        "trn_boot_mod", "/root/.axon_site/trn_agent_boot/trn_boot.py")
    tb = ilu.module_from_spec(spec)
    spec.loader.exec_module(tb)
    hook = tb._ntff_profile_via_ctypes("/opt/axon/libaxon_pjrt.so")
    mod = types.ModuleType("antenv.axon_hooks")
    mod.get_axon_ntff_profile_hook = lambda: hook
    import antenv  # noqa: F401
    sys.modules["antenv.axon_hooks"] = mod


def kernel(**inputs) -> np.ndarray:
    in_maps = _prep_inputs(**inputs)
    r = _run(in_maps)
    outs = [np.ascontiguousarray(
        r.results[c]["out"].transpose(0, 2, 1)).astype(np.float32)
        for c in range(NCORES)]
    return np.concatenate(outs, axis=0)
